# revision 14
# baseline (speedup 1.0000x reference)
"""Trainium2 Bass kernel for nn_EnhancedDistillationLoss.

Distillation loss = CE_W * masked-CE(student_logits, labels)
                  + KL_W * masked-KL(uniform-teacher || student @ TEMP)

Strategy (data parallel over the 8 NeuronCores):
  - Flatten logits to [B*S, V] = [1024, 151643] rows; core c owns rows
    [128c, 128c+128) -> 128 rows = 128 SBUF partitions, vocab on the free
    axis, streamed in tiles of TILE_W.
  - Per tile (per partition/row r):
      ACT    : y = exp(0.5*x) (bf16) with accum_out -> S2 += sum(exp(x/2))
      DVE    : tensor_tensor_reduce y*y -> S1 += sum(exp(x))   (dummy out)
      PE     : f32r matmul w[128,1]^T @ x[:,chunk] accumulated in one PSUM
               bank -> Wx = sum_r w_r * sum_v x[r, v]  (w = mask * p)
      (DMA streams the next tile meanwhile; kernel is HBM-bandwidth-bound)
  - One indirect DMA gathers x[r, label_r] per row.
  - Host combines per-core scalars exactly like the reference:
      logsumexp(x)   = log(S1)          (no max-sub needed: |x| ~ N(0,1))
      logsumexp(x/2) = log(S2)
      ce  = mean_valid(lse1 - x[label])
      slp_sum        = sum(x)/2 - V*lse2   (weighted form via Wx)
      kl  = mean_mask(V*p*log p - p*slp_sum) * TEMP^2
"""

import functools
import os
from contextlib import ExitStack

import numpy as np

import concourse.bacc as bacc
import concourse.tile as tile
from concourse import bass, mybir
from concourse.bass_utils import run_bass_kernel_spmd

B, S, V = 2, 512, 151643
TEMP = 2.0
CE_W, KL_W = 1.0, 0.5
N_CORES = 8
P = 128  # rows per core == SBUF partitions
TILE_W = 8192  # vocab tile width (fp32: 32KB/partition)
MM_N = 512  # matmul moving free dim (one PSUM bank)

f32 = mybir.dt.float32
f32r = mybir.dt.float32r
bf16 = mybir.dt.bfloat16
i32 = mybir.dt.int32


def _ceil_div(a, b):
    return -(-a // b)


GATHER_BLK = 64  # indirect-DMA gather granularity (64 f32 = 256 B)


def build_kernel(v=V, tile_w=TILE_W, p=P):
    nc = bacc.Bacc("TRN2", target_bir_lowering=False, debug=False)
    x = nc.dram_tensor("x", [p, v], f32, kind="ExternalInput")
    gidx = nc.dram_tensor("gidx", [p, 1], i32, kind="ExternalInput")
    onehot = nc.dram_tensor("onehot", [p, GATHER_BLK], f32, kind="ExternalInput")
    stats = nc.dram_tensor("stats", [p, 4], f32, kind="ExternalOutput")

    n_tiles = _ceil_div(v, tile_w)

    with TileContextWrapper(nc) as (tc, ctx):
        xp = ctx.enter_context(tc.tile_pool(name="xp", bufs=3))
        yp = ctx.enter_context(tc.tile_pool(name="yp", bufs=2))
        accp = ctx.enter_context(tc.tile_pool(name="accp", bufs=1))

        s1p = accp.tile([p, n_tiles], f32)
        s2p = accp.tile([p, n_tiles], f32)
        txp = accp.tile([p, n_tiles], f32)
        sq_dummy = accp.tile([p, 1], bf16)
        ts_dummy = accp.tile([p, 1], f32)
        idx_sb = accp.tile([p, 1], i32)
        oh_sb = accp.tile([p, GATHER_BLK], f32)
        blk_sb = accp.tile([p, GATHER_BLK], f32)
        blk_dummy = accp.tile([p, 1], f32)
        stats_sb = accp.tile([p, 4], f32)

        # gather: stats col 3 <- x[r, label_r] via a 256B-aligned block
        # indirect DMA + one-hot dot (single-element indirect DMA faults).
        nc.sync.dma_start(out=idx_sb[:], in_=gidx[:])
        nc.sync.dma_start(out=oh_sb[:], in_=onehot[:])
        nc.gpsimd.indirect_dma_start(
            out=blk_sb[:],
            out_offset=None,
            in_=x[:]
            .rearrange("p v -> (p v)")
            .rearrange("(a b) -> a b", b=GATHER_BLK),
            in_offset=bass.IndirectOffsetOnAxis(ap=idx_sb[:, :1], axis=0),
        )
        nc.vector.scalar_tensor_tensor(
            out=blk_dummy[:].broadcast_to((p, GATHER_BLK)),
            in0=blk_sb[:],
            scalar=1.0,
            in1=oh_sb[:],
            op0=mybir.AluOpType.mult,
            op1=mybir.AluOpType.mult,
            accum_out=stats_sb[:, 3:4],
        )

        for t in range(n_tiles):
            w0 = t * tile_w
            wt = min(tile_w, v - w0)
            xt = xp.tile([p, tile_w], f32, tag="x")
            yt = yp.tile([p, tile_w], bf16, tag="y")
            nc.sync.dma_start(out=xt[:, :wt], in_=x[:, w0 : w0 + wt])
            nc.scalar.activation(
                out=yt[:, :wt],
                in_=xt[:, :wt],
                func=mybir.ActivationFunctionType.Exp,
                scale=0.5,
                accum_out=s2p[:, t : t + 1],
            )
            # S1 partial: sum(y*y) = sum(exp(x))
            nc.vector.affine_mul_reduce(
                out=sq_dummy[:].broadcast_to((p, wt)),
                accum_out=s1p[:, t : t + 1],
                in0=yt[:, :wt],
                in1=yt[:, :wt],
                scale=1.0,
                bias=0.0,
            )
            # T partial: sum(x)
            nc.vector.tensor_scalar(
                out=ts_dummy[:].broadcast_to((p, wt)),
                in0=xt[:, :wt],
                scalar1=1.0,
                scalar2=0.0,
                op0=mybir.AluOpType.mult,
                op1=mybir.AluOpType.add,
                accum_out=txp[:, t : t + 1],
            )

        nc.vector.reduce_sum(
            out=stats_sb[:, 0:1], in_=s1p[:], axis=mybir.AxisListType.X
        )
        nc.vector.reduce_sum(
            out=stats_sb[:, 1:2], in_=s2p[:], axis=mybir.AxisListType.X
        )
        nc.vector.reduce_sum(
            out=stats_sb[:, 2:3], in_=txp[:], axis=mybir.AxisListType.X
        )
        nc.sync.dma_start(out=stats[:], in_=stats_sb[:])
    nc.compile()
    return nc


class TileContextWrapper:
    """TileContext + ExitStack in one `with`."""

    def __init__(self, nc):
        self.nc = nc

    def __enter__(self):
        self.ctx = ExitStack()
        self.ctx.__enter__()
        self.tc = tile.TileContext(self.nc)
        self.tc.__enter__()
        return self.tc, self.ctx

    def __exit__(self, *exc):
        # close pools before TileContext exit (scheduling)
        self.ctx.__exit__(*exc)
        return self.tc.__exit__(*exc)


@functools.lru_cache(maxsize=1)
def _get_nc():
    return build_kernel()


def host_combine(stats, labels_flat, mask_flat, p_row):
    """Combine per-row device sums into the final scalar loss (float64)."""
    S1 = stats[:, 0].astype(np.float64)
    S2 = stats[:, 1].astype(np.float64)
    T = stats[:, 2].astype(np.float64)
    g = stats[:, 3].astype(np.float64)
    lse1 = np.log(S1)  # logsumexp(x) per row
    lse2 = np.log(S2)  # logsumexp(x/2) per row
    valid = labels_flat != -100
    n_valid = max(int(valid.sum()), 1)
    ce = float(np.sum(np.where(valid, lse1 - g, 0.0)) / n_valid)

    slp_sum = 0.5 * T - V * lse2  # sum_v log_softmax(x/2) per row
    logp = np.log(p_row)
    kl_token = V * p_row * logp - p_row * slp_sum
    kl_sum = float(np.sum(mask_flat * kl_token))
    msum = float(mask_flat.sum())
    kl = (kl_sum / msum if msum > 0 else kl_sum) * (TEMP**2)
    return CE_W * ce + KL_W * kl


def kernel(student_logits, teacher_token_logprobs, labels, attention_mask):
    x2d = np.ascontiguousarray(
        np.asarray(student_logits, dtype=np.float32).reshape(B * S, V)
    )
    labels_flat = np.asarray(labels).reshape(-1).astype(np.int64)
    mask_flat = np.asarray(attention_mask).reshape(-1).astype(np.float64)
    tlp = np.asarray(teacher_token_logprobs, dtype=np.float64)
    prob = np.minimum(np.exp(tlp), 0.99)
    p_t = (1.0 - prob) / V  # [S]
    p_row = np.tile(p_t, B)  # [B*S] row-major (b, t)
    safe_labels = np.where(labels_flat < 0, 0, labels_flat)

    nc = _get_nc()
    in_maps = []
    for c in range(N_CORES):
        sl = slice(c * P, (c + 1) * P)
        flat = np.arange(P, dtype=np.int64) * V + safe_labels[sl]
        g_idx = (flat // GATHER_BLK).astype(np.int32)
        onehot = np.zeros((P, GATHER_BLK), dtype=np.float32)
        onehot[np.arange(P), flat % GATHER_BLK] = 1.0
        in_maps.append({"x": x2d[sl], "gidx": g_idx[:, None], "onehot": onehot})
    trace = bool(int(os.environ.get("KERNEL_TRACE", "0")))
    res = run_bass_kernel_spmd(
        nc, in_maps, core_ids=list(range(N_CORES)), trace=trace
    )
    global _LAST_RESULTS
    _LAST_RESULTS = res
    stats = np.concatenate([r["stats"] for r in res.results], axis=0)
    total = host_combine(stats, labels_flat, mask_flat, p_row)
    return np.float32(total)


_LAST_RESULTS = None


# revision 21
# speedup vs baseline: 379.5495x; 379.5495x over previous
"""Trainium2 Bass kernel for nn_EnhancedDistillationLoss.

Distillation loss = CE_W * masked-CE(student_logits, labels)
                  + KL_W * masked-KL(uniform-teacher || student @ TEMP)

Strategy (data parallel over the 8 NeuronCores):
  - Flatten logits to [B*S, V] = [1024, 151643] rows; core c owns rows
    [128c, 128c+128) -> 128 rows = 128 SBUF partitions, vocab on the free
    axis, streamed in tiles of TILE_W.
  - Per tile (per partition/row r):
      ACT    : y = exp(0.5*x) (bf16) with accum_out -> S2 += sum(exp(x/2))
      DVE    : tensor_tensor_reduce y*y -> S1 += sum(exp(x))   (dummy out)
      PE     : f32r matmul w[128,1]^T @ x[:,chunk] accumulated in one PSUM
               bank -> Wx = sum_r w_r * sum_v x[r, v]  (w = mask * p)
      (DMA streams the next tile meanwhile; kernel is HBM-bandwidth-bound)
  - One indirect DMA gathers x[r, label_r] per row.
  - Host combines per-core scalars exactly like the reference:
      logsumexp(x)   = log(S1)          (no max-sub needed: |x| ~ N(0,1))
      logsumexp(x/2) = log(S2)
      ce  = mean_valid(lse1 - x[label])
      slp_sum        = sum(x)/2 - V*lse2   (weighted form via Wx)
      kl  = mean_mask(V*p*log p - p*slp_sum) * TEMP^2
"""

import functools
import os
from contextlib import ExitStack

import numpy as np

import concourse.bacc as bacc
import concourse.tile as tile
from concourse import bass, mybir
from concourse.bass_utils import run_bass_kernel_spmd

B, S, V = 2, 512, 151643
TEMP = 2.0
CE_W, KL_W = 1.0, 0.5
N_CORES = 8
P = 128  # rows per core == SBUF partitions
TILE_W = 8192  # vocab tile width (fp32: 32KB/partition, 4MB per DMA)
X_BUFS = 4
Y_BUFS = 3
# Fraction of tiles whose sum-of-squares runs on ACT (Square) instead of
# the Vector engine, to balance the two engines: tile t -> ACT iff
# (t * BACT_NUM) % BACT_DEN < BACT_NUM.
BACT_NUM, BACT_DEN = 5, 9

f32 = mybir.dt.float32
f32r = mybir.dt.float32r
bf16 = mybir.dt.bfloat16
i32 = mybir.dt.int32


def _ceil_div(a, b):
    return -(-a // b)


GATHER_BLK = 64  # indirect-DMA gather granularity (64 f32 = 256 B)


def build_kernel(v=V, tile_w=TILE_W, p=P):
    nc = bacc.Bacc("TRN2", target_bir_lowering=False, debug=False)
    x = nc.dram_tensor("x", [p, v], f32, kind="ExternalInput")
    gidx = nc.dram_tensor("gidx", [p, 1], i32, kind="ExternalInput")
    onehot = nc.dram_tensor("onehot", [p, GATHER_BLK], f32, kind="ExternalInput")
    stats = nc.dram_tensor("stats", [p, 4], f32, kind="ExternalOutput")

    n_tiles = _ceil_div(v, tile_w)

    with TileContextWrapper(nc) as (tc, ctx):
        xp = ctx.enter_context(tc.tile_pool(name="xp", bufs=X_BUFS))
        yp = ctx.enter_context(tc.tile_pool(name="yp", bufs=Y_BUFS))
        accp = ctx.enter_context(tc.tile_pool(name="accp", bufs=1))

        s1p = accp.tile([p, n_tiles], f32)
        s2p = accp.tile([p, n_tiles], f32)
        txp = accp.tile([p, n_tiles], f32)
        sq_dummy = accp.tile([p, 1], bf16)
        sq_dummy_act = accp.tile([p, 1], bf16)
        ts_dummy = accp.tile([p, 1], f32)
        idx_sb = accp.tile([p, 1], i32)
        oh_sb = accp.tile([p, GATHER_BLK], f32)
        blk_sb = accp.tile([p, GATHER_BLK], f32)
        blk_dummy = accp.tile([p, 1], f32)
        stats_sb = accp.tile([p, 4], f32)

        # gather: stats col 3 <- x[r, label_r] via a 256B-aligned block
        # indirect DMA + one-hot dot (single-element indirect DMA faults).
        nc.sync.dma_start(out=idx_sb[:], in_=gidx[:])
        nc.sync.dma_start(out=oh_sb[:], in_=onehot[:])
        nc.gpsimd.indirect_dma_start(
            out=blk_sb[:],
            out_offset=None,
            in_=x[:]
            .rearrange("p v -> (p v)")
            .rearrange("(a b) -> a b", b=GATHER_BLK),
            in_offset=bass.IndirectOffsetOnAxis(ap=idx_sb[:, :1], axis=0),
        )
        nc.vector.scalar_tensor_tensor(
            out=blk_dummy[:].broadcast_to((p, GATHER_BLK)),
            in0=blk_sb[:],
            scalar=1.0,
            in1=oh_sb[:],
            op0=mybir.AluOpType.mult,
            op1=mybir.AluOpType.mult,
            accum_out=stats_sb[:, 3:4],
        )

        for t in range(n_tiles):
            w0 = t * tile_w
            wt = min(tile_w, v - w0)
            xt = xp.tile([p, tile_w], f32, tag="x")
            yt = yp.tile([p, tile_w], bf16, tag="y")
            nc.sync.dma_start(out=xt[:, :wt], in_=x[:, w0 : w0 + wt])
            nc.scalar.activation(
                out=yt[:, :wt],
                in_=xt[:, :wt],
                func=mybir.ActivationFunctionType.Exp,
                scale=0.5,
                accum_out=s2p[:, t : t + 1],
            )
            # S1 partial: sum(y*y) = sum(exp(x)). Load-balanced between the
            # Vector engine (affine_mul_reduce, 1x) and the Scalar engine
            # (Square + accum; same table set as Exp, so no table reload).
            if (t * BACT_NUM) % BACT_DEN < BACT_NUM:
                nc.scalar.activation(
                    out=sq_dummy_act[:].broadcast_to((p, wt)),
                    in_=yt[:, :wt],
                    func=mybir.ActivationFunctionType.Square,
                    accum_out=s1p[:, t : t + 1],
                )
            else:
                nc.vector.affine_mul_reduce(
                    out=sq_dummy[:].broadcast_to((p, wt)),
                    accum_out=s1p[:, t : t + 1],
                    in0=yt[:, :wt],
                    in1=yt[:, :wt],
                    scale=1.0,
                    bias=0.0,
                )
            # T partial: sum(x)
            nc.vector.tensor_scalar(
                out=ts_dummy[:].broadcast_to((p, wt)),
                in0=xt[:, :wt],
                scalar1=1.0,
                scalar2=0.0,
                op0=mybir.AluOpType.mult,
                op1=mybir.AluOpType.add,
                accum_out=txp[:, t : t + 1],
            )

        nc.vector.reduce_sum(
            out=stats_sb[:, 0:1], in_=s1p[:], axis=mybir.AxisListType.X
        )
        nc.vector.reduce_sum(
            out=stats_sb[:, 1:2], in_=s2p[:], axis=mybir.AxisListType.X
        )
        nc.vector.reduce_sum(
            out=stats_sb[:, 2:3], in_=txp[:], axis=mybir.AxisListType.X
        )
        nc.sync.dma_start(out=stats[:], in_=stats_sb[:])
    nc.compile()
    return nc


class TileContextWrapper:
    """TileContext + ExitStack in one `with`."""

    def __init__(self, nc):
        self.nc = nc

    def __enter__(self):
        self.ctx = ExitStack()
        self.ctx.__enter__()
        self.tc = tile.TileContext(self.nc)
        self.tc.__enter__()
        return self.tc, self.ctx

    def __exit__(self, *exc):
        # close pools before TileContext exit (scheduling)
        self.ctx.__exit__(*exc)
        return self.tc.__exit__(*exc)


@functools.lru_cache(maxsize=1)
def _get_nc():
    return build_kernel()


def host_combine(stats, labels_flat, mask_flat, p_row):
    """Combine per-row device sums into the final scalar loss (float64)."""
    S1 = stats[:, 0].astype(np.float64)
    S2 = stats[:, 1].astype(np.float64)
    T = stats[:, 2].astype(np.float64)
    g = stats[:, 3].astype(np.float64)
    lse1 = np.log(S1)  # logsumexp(x) per row
    lse2 = np.log(S2)  # logsumexp(x/2) per row
    valid = labels_flat != -100
    n_valid = max(int(valid.sum()), 1)
    ce = float(np.sum(np.where(valid, lse1 - g, 0.0)) / n_valid)

    slp_sum = 0.5 * T - V * lse2  # sum_v log_softmax(x/2) per row
    logp = np.log(p_row)
    kl_token = V * p_row * logp - p_row * slp_sum
    kl_sum = float(np.sum(mask_flat * kl_token))
    msum = float(mask_flat.sum())
    kl = (kl_sum / msum if msum > 0 else kl_sum) * (TEMP**2)
    return CE_W * ce + KL_W * kl


def kernel(student_logits, teacher_token_logprobs, labels, attention_mask):
    x2d = np.ascontiguousarray(
        np.asarray(student_logits, dtype=np.float32).reshape(B * S, V)
    )
    labels_flat = np.asarray(labels).reshape(-1).astype(np.int64)
    mask_flat = np.asarray(attention_mask).reshape(-1).astype(np.float64)
    tlp = np.asarray(teacher_token_logprobs, dtype=np.float64)
    prob = np.minimum(np.exp(tlp), 0.99)
    p_t = (1.0 - prob) / V  # [S]
    p_row = np.tile(p_t, B)  # [B*S] row-major (b, t)
    safe_labels = np.where(labels_flat < 0, 0, labels_flat)

    nc = _get_nc()
    in_maps = []
    for c in range(N_CORES):
        sl = slice(c * P, (c + 1) * P)
        flat = np.arange(P, dtype=np.int64) * V + safe_labels[sl]
        g_idx = (flat // GATHER_BLK).astype(np.int32)
        onehot = np.zeros((P, GATHER_BLK), dtype=np.float32)
        onehot[np.arange(P), flat % GATHER_BLK] = 1.0
        in_maps.append({"x": x2d[sl], "gidx": g_idx[:, None], "onehot": onehot})
    trace = bool(int(os.environ.get("KERNEL_TRACE", "0")))
    res = run_bass_kernel_spmd(
        nc, in_maps, core_ids=list(range(N_CORES)), trace=trace
    )
    global _LAST_RESULTS
    _LAST_RESULTS = res
    stats = np.concatenate([r["stats"] for r in res.results], axis=0)
    total = host_combine(stats, labels_flat, mask_flat, p_row)
    return np.float32(total)


_LAST_RESULTS = None
